# revision 1
# baseline (speedup 1.0000x reference)
"""DeepSeekV3 MLA attention kernel for Trainium2 (8 NeuronCores, Bass/Tile).

Sharding (no collectives): core c -> batch b = c // 4, head-group g = c % 4
(8 of the 32 heads).  Each core runs the full layer for its batch restricted
to its heads and emits a partial o_proj output [2048, 4096]; the host sums
the 4 partials per batch.  The small a-projections (q_a, kv_a) are
recomputed per core (replicated 4x within a batch).

Layouts (feature-major, [128, chunks, tokens]):
  - x is fed transposed (xT [4096, 2048]); all matmuls contract over the
    partition dim with N = 512 token tiles (one PSUM bank, full fp32r rate).
  - RoPE halves are packed [4*lo(128) | 4*hi(128)] per 4 heads so the rotate
    is partition-aligned full-lane DVE work; a DMA rearrange then stores the
    per-head-contiguous [64] blocks that attention contracts over (K=64).
  - Softmax skips the max-subtraction (scores are O(5) here, exp is safe in
    fp32); denominators come from an all-ones matmul accumulated alongside
    the PV matmul, so the whole attention inner loop is 4 matmuls + 1 exp.

All matmul operands are declared float32r (TF32-like, ~1.6e-4 rel error,
full PE rate at N >= 256); PSUM accumulation stays fp32.
"""

import math

import numpy as np

try:
    import concourse.bacc as bacc  # noqa: F401
except ImportError:
    import sys

    for _p in ("/root/.axon_site/_ro/trn_rl_repo", "/opt/trn_rl_repo"):
        if _p not in sys.path:
            sys.path.insert(0, _p)

import concourse.bacc as bacc
import concourse.mybir as mybir
import concourse.tile as tile
from concourse.bass_utils import run_bass_kernel_spmd

# model dims
H, DN, DR, DV = 32, 128, 64, 128
HID, QR, KVR = 4096, 1536, 512
EPS, MAXP = 1e-6, 4096
B, S = 2, 2048
P = 128
TT = 512  # token tile (matmul moving dim)
NH = 8  # heads per core
NCORES = 8
SCALE = 1.0 / math.sqrt(DN + DR)
HIDC = HID // P  # 32
QRC = QR // P  # 12
KVRC = KVR // P  # 4

F32 = mybir.dt.float32
F32R = mybir.dt.float32r

EXP_FN = mybir.ActivationFunctionType.Exp
SQRT_FN = mybir.ActivationFunctionType.Sqrt


def build_nc(tb=S):
    """Build the per-core Bass program (same program on all 8 cores)."""
    ntt = tb // TT
    ntc = tb // P  # token chunks
    nc = bacc.Bacc("TRN2", target_bir_lowering=False, debug=False)

    xT = nc.dram_tensor("xT", [HID, tb], F32R, kind="ExternalInput")
    qa_wT = nc.dram_tensor("qa_wT", [HID, QR], F32R, kind="ExternalInput")
    kva_wT = nc.dram_tensor("kva_wT", [HID, KVR], F32R, kind="ExternalInput")
    kr_wT = nc.dram_tensor("kr_wT", [HID, NH * DR], F32R, kind="ExternalInput")
    qb_wT = nc.dram_tensor("qb_wT", [QR, NH * (DN + DR)], F32R, kind="ExternalInput")
    kvbk_wT = nc.dram_tensor("kvbk_wT", [KVR, NH * DN], F32R, kind="ExternalInput")
    kvbv_wT = nc.dram_tensor("kvbv_wT", [KVR, NH * DV], F32R, kind="ExternalInput")
    o_wT = nc.dram_tensor("o_wT", [NH * DV, HID], F32R, kind="ExternalInput")
    cos_in = nc.dram_tensor("cos_rep", [P, tb], F32, kind="ExternalInput")
    sin_in = nc.dram_tensor("sin_rep", [P, tb], F32, kind="ExternalInput")
    out_part = nc.dram_tensor("out_part", [tb, HID], F32, kind="ExternalOutput")

    x_ap = xT[:, :].rearrange("(c p) t -> p c t", p=P)
    qa_ap = qa_wT[:, :].rearrange("(c p) m -> p c m", p=P)
    kva_ap = kva_wT[:, :].rearrange("(c p) m -> p c m", p=P)
    kr_ap = kr_wT[:, :].rearrange("(c p) m -> p c m", p=P)
    qb_ap = qb_wT[:, :].rearrange("(c p) m -> p c m", p=P)
    kvbk_ap = kvbk_wT[:, :].rearrange("(c p) m -> p c m", p=P)
    kvbv_ap = kvbv_wT[:, :].rearrange("(c p) m -> p c m", p=P)
    ow_ap = o_wT[:, :].rearrange("(c p) m -> p c m", p=P)

    with tile.TileContext(nc) as tc:
        with tc.tile_pool(name="const", bufs=1) as constp, \
             tc.tile_pool(name="dram", bufs=1, space="DRAM") as dram:
            ones_f = constp.tile([P, P], F32)
            nc.any.memset(ones_f[:], 1.0)
            ones_r = constp.tile([P, P], F32R)
            nc.vector.tensor_copy(out=ones_r[:], in_=ones_f[:])
            eps_sb = constp.tile([P, 1], F32)
            nc.any.memset(eps_sb[:], EPS)
            cos_sb = constp.tile([P, tb], F32)
            sin_sb = constp.tile([P, tb], F32)
            nc.sync.dma_start(out=cos_sb[:], in_=cos_in[:, :])
            nc.sync.dma_start(out=sin_sb[:], in_=sin_in[:, :])

            qlat_d = dram.tile([P, QRC, tb], F32R)
            kvlat_d = dram.tile([P, KVRC, tb], F32R)
            rstdq_d = dram.tile([P, tb], F32)
            rstdkv_d = dram.tile([P, tb], F32)
            qnope_d = dram.tile([P, NH, tb], F32R)
            qrope_d = dram.tile([P, NH * DR // P, tb], F32R)
            knope_d = dram.tile([P, NH, tb], F32R)
            krope_d = dram.tile([P, NH * DR // P, tb], F32R)
            v_d = dram.tile([P, ntc, NH * DV], F32R)
            attn_d = dram.tile([P, NH, tb], F32R)

            def rope_evict(lo_src, hi_src, dst_d, tsl, pool, tag):
                """lo/hi chunk pair [P, TT] (4 heads x 32 rows) -> rotate ->
                per-head-contiguous [64] blocks in dst_d."""
                t1 = pool.tile([P, TT], F32, tag=tag, name="rt1")
                t2 = pool.tile([P, TT], F32, tag=tag, name="rt2")
                nc.vector.tensor_mul(out=t1[:], in0=lo_src[:], in1=cos_sb[:, tsl])
                nc.vector.tensor_mul(out=t2[:], in0=hi_src[:], in1=sin_sb[:, tsl])
                lo_o = pool.tile([P, TT], F32R, tag=tag, name="rlo")
                nc.vector.tensor_sub(out=lo_o[:], in0=t1[:], in1=t2[:])
                t3 = pool.tile([P, TT], F32, tag=tag, name="rt3")
                t4 = pool.tile([P, TT], F32, tag=tag, name="rt4")
                nc.vector.tensor_mul(out=t3[:], in0=hi_src[:], in1=cos_sb[:, tsl])
                nc.vector.tensor_mul(out=t4[:], in0=lo_src[:], in1=sin_sb[:, tsl])
                hi_o = pool.tile([P, TT], F32R, tag=tag, name="rhi")
                nc.vector.tensor_add(out=hi_o[:], in0=t3[:], in1=t4[:])
                return lo_o, hi_o

            def rope_store(lo_o, hi_o, ci, dst_d, tsl):
                for hh in range(4):
                    h = ci * 4 + hh
                    dc, dp = h // 2, 64 * (h % 2)
                    nc.sync.dma_start(
                        out=dst_d[dp:dp + 32, dc, tsl],
                        in_=lo_o[32 * hh:32 * hh + 32, :])
                    nc.sync.dma_start(
                        out=dst_d[dp + 32:dp + 64, dc, tsl],
                        in_=hi_o[32 * hh:32 * hh + 32, :])

            # ---------------- Phase A: fused a-projections + k_rope --------
            # groups of output chunks; contraction over HID (32 k-chunks)
            groups = [("q", 0, 6), ("q", 6, 12), ("kv", 0, 4), ("kr", 0, 4)]
            srcs = {"q": qa_ap, "kv": kva_ap, "kr": kr_ap}
            with tc.tile_pool(name="apw", bufs=7) as wpool, \
                 tc.tile_pool(name="apx", bufs=4) as xpool, \
                 tc.tile_pool(name="apev", bufs=10) as evp, \
                 tc.tile_pool(name="apss", bufs=5) as ssqp, \
                 tc.tile_pool(name="apacc", bufs=6, space="PSUM") as accp, \
                 tc.tile_pool(name="apstat", bufs=2, space="PSUM") as statp:
                qssq0 = {}
                for gi, (kind, m0, m1) in enumerate(groups):
                    src = srcs[kind]
                    wts = []
                    for m in range(m0, m1):
                        wt = wpool.tile([P, HIDC, P], F32R, tag="apw",
                                        name=f"apw{gi}_{m}")
                        nc.sync.dma_start(out=wt[:], in_=src[:, :, m * P:(m + 1) * P])
                        wts.append(wt)
                    for t in range(ntt):
                        tsl = slice(t * TT, (t + 1) * TT)
                        accs = [accp.tile([P, TT], F32, tag="acc",
                                          name=f"acc{gi}_{t}_{m}")
                                for m in range(m0, m1)]
                        for k in range(HIDC):
                            xt = xpool.tile([P, TT], F32R, tag="apx",
                                            name=f"x{gi}_{t}_{k}")
                            nc.sync.dma_start(out=xt[:], in_=x_ap[:, k, tsl])
                            for mi in range(m1 - m0):
                                nc.tensor.matmul(
                                    accs[mi][:], wts[mi][:, k, :], xt[:],
                                    start=(k == 0), stop=(k == HIDC - 1))
                        if kind == "q":
                            stat = statp.tile([P, TT], F32, tag="stat",
                                              name=f"stat{gi}_{t}")
                            for mi, m in enumerate(range(m0, m1)):
                                raw = evp.tile([P, TT], F32R, tag="ev")
                                nc.vector.tensor_copy(out=raw[:], in_=accs[mi][:])
                                nc.sync.dma_start(out=qlat_d[:, m, tsl], in_=raw[:])
                                sq = evp.tile([P, TT], F32R, tag="ev")
                                nc.vector.tensor_mul(out=sq[:], in0=raw[:],
                                                     in1=raw[:])
                                nc.tensor.matmul(stat[:], ones_r[:], sq[:],
                                                 start=(mi == 0),
                                                 stop=(mi == m1 - m0 - 1))
                            if gi == 0:
                                part = ssqp.tile([P, TT], F32, tag="qssq",
                                                 name=f"qssq_{t}")
                                nc.vector.tensor_copy(out=part[:], in_=stat[:])
                                qssq0[t] = part
                            else:
                                ssq = evp.tile([P, TT], F32, tag="ev")
                                nc.vector.tensor_add(out=ssq[:], in0=stat[:],
                                                     in1=qssq0[t][:])
                                sdev = evp.tile([P, TT], F32, tag="ev")
                                nc.scalar.activation(sdev[:], ssq[:], SQRT_FN,
                                                     bias=eps_sb[:],
                                                     scale=1.0 / QR)
                                rstd = evp.tile([P, TT], F32, tag="ev")
                                nc.vector.reciprocal(rstd[:], sdev[:])
                                nc.sync.dma_start(out=rstdq_d[:, tsl], in_=rstd[:])
                        elif kind == "kv":
                            stat = statp.tile([P, TT], F32, tag="stat",
                                              name=f"statkv_{t}")
                            for mi, m in enumerate(range(m0, m1)):
                                raw = evp.tile([P, TT], F32R, tag="ev")
                                nc.vector.tensor_copy(out=raw[:], in_=accs[mi][:])
                                nc.sync.dma_start(out=kvlat_d[:, m, tsl], in_=raw[:])
                                sq = evp.tile([P, TT], F32R, tag="ev")
                                nc.vector.tensor_mul(out=sq[:], in0=raw[:],
                                                     in1=raw[:])
                                nc.tensor.matmul(stat[:], ones_r[:], sq[:],
                                                 start=(mi == 0),
                                                 stop=(mi == m1 - m0 - 1))
                            sdev = evp.tile([P, TT], F32, tag="ev")
                            nc.scalar.activation(sdev[:], stat[:], SQRT_FN,
                                                 bias=eps_sb[:],
                                                 scale=1.0 / KVR)
                            rstd = evp.tile([P, TT], F32, tag="ev")
                            nc.vector.reciprocal(rstd[:], sdev[:])
                            nc.sync.dma_start(out=rstdkv_d[:, tsl], in_=rstd[:])
                        else:  # kr: chunks [lo0, lo1, hi0, hi1] -> rope
                            for ci in range(2):
                                lo_o, hi_o = rope_evict(
                                    accs[ci], accs[2 + ci], krope_d, tsl, evp, "ev")
                                rope_store(lo_o, hi_o, ci, krope_d, tsl)

            # ---------------- Phase B: q_b + q rope -------------------------
            with tc.tile_pool(name="qbw", bufs=1) as qbwp, \
                 tc.tile_pool(name="qbn", bufs=QRC + 1) as qnp, \
                 tc.tile_pool(name="qbio", bufs=3) as iop, \
                 tc.tile_pool(name="qbev", bufs=10) as evp, \
                 tc.tile_pool(name="qbps", bufs=6, space="PSUM") as qbps:
                qbw = qbwp.tile([P, QRC, NH * (DN + DR)], F32R)
                nc.sync.dma_start(out=qbw[:], in_=qb_ap[:, :, :])
                for t in range(ntt):
                    tsl = slice(t * TT, (t + 1) * TT)
                    rstd = iop.tile([P, TT], F32, tag="ev")
                    nc.sync.dma_start(out=rstd[:], in_=rstdq_d[:, tsl])
                    qn = []
                    for k in range(QRC):
                        raw = iop.tile([P, TT], F32R, tag="qraw")
                        nc.sync.dma_start(out=raw[:], in_=qlat_d[:, k, tsl])
                        qnk = qnp.tile([P, TT], F32R, tag="qn", name=f"qn{t}_{k}")
                        nc.vector.tensor_mul(out=qnk[:], in0=raw[:], in1=rstd[:])
                        qn.append(qnk)
                    rope_ps = {}
                    for m in range(QRC):
                        ps = qbps.tile([P, TT], F32, tag="qbps", name=f"qb{t}_{m}")
                        for k in range(QRC):
                            nc.tensor.matmul(ps[:], qbw[:, k, m * P:(m + 1) * P],
                                             qn[k][:], start=(k == 0),
                                             stop=(k == QRC - 1))
                        if m < NH:
                            o = evp.tile([P, TT], F32R, tag="ev")
                            nc.vector.tensor_copy(out=o[:], in_=ps[:])
                            nc.sync.dma_start(out=qnope_d[:, m, tsl], in_=o[:])
                        else:
                            rope_ps[m - NH] = ps
                    for ci in range(2):
                        lo_o, hi_o = rope_evict(rope_ps[ci], rope_ps[2 + ci],
                                                qrope_d, tsl, evp, "ev")
                        rope_store(lo_o, hi_o, ci, qrope_d, tsl)

            # ---------------- Phase C: kv_b (k_nope + v) --------------------
            with tc.tile_pool(name="kvw", bufs=1) as kvwp, \
                 tc.tile_pool(name="kvn", bufs=KVRC + 1) as kvnp, \
                 tc.tile_pool(name="kvio", bufs=3) as iop, \
                 tc.tile_pool(name="kvev", bufs=4) as evp, \
                 tc.tile_pool(name="kvps", bufs=3, space="PSUM") as kvps:
                kbw = kvwp.tile([P, KVRC, NH * DN], F32R)
                vbw = kvwp.tile([P, KVRC, NH * DV], F32R)
                nc.sync.dma_start(out=kbw[:], in_=kvbk_ap[:, :, :])
                nc.sync.dma_start(out=vbw[:], in_=kvbv_ap[:, :, :])
                for t in range(ntt):
                    tsl = slice(t * TT, (t + 1) * TT)
                    rstd = iop.tile([P, TT], F32, tag="ev")
                    nc.sync.dma_start(out=rstd[:], in_=rstdkv_d[:, tsl])
                    kvn = []
                    for k in range(KVRC):
                        raw = iop.tile([P, TT], F32R, tag="kvraw")
                        nc.sync.dma_start(out=raw[:], in_=kvlat_d[:, k, tsl])
                        kk = kvnp.tile([P, TT], F32R, tag="kvn", name=f"kvn{t}_{k}")
                        nc.vector.tensor_mul(out=kk[:], in0=raw[:], in1=rstd[:])
                        kvn.append(kk)
                    for m in range(NH):
                        ps = kvps.tile([P, TT], F32, tag="kps", name=f"kb{t}_{m}")
                        for k in range(KVRC):
                            nc.tensor.matmul(ps[:], kbw[:, k, m * P:(m + 1) * P],
                                             kvn[k][:], start=(k == 0),
                                             stop=(k == KVRC - 1))
                        o = evp.tile([P, TT], F32R, tag="ev")
                        nc.vector.tensor_copy(out=o[:], in_=ps[:])
                        nc.sync.dma_start(out=knope_d[:, m, tsl], in_=o[:])
                    for tc8 in range(TT // P):
                        for vc in range(NH * DV // TT):
                            ps = kvps.tile([P, TT], F32, tag="vps",
                                           name=f"v{t}_{tc8}_{vc}")
                            for k in range(KVRC):
                                nc.tensor.matmul(
                                    ps[:],
                                    kvn[k][:, tc8 * P:(tc8 + 1) * P],
                                    vbw[:, k, vc * TT:(vc + 1) * TT],
                                    start=(k == 0), stop=(k == KVRC - 1))
                            o = evp.tile([P, TT], F32R, tag="ev")
                            nc.vector.tensor_copy(out=o[:], in_=ps[:])
                            nc.sync.dma_start(
                                out=v_d[:, t * (TT // P) + tc8,
                                        vc * TT:(vc + 1) * TT],
                                in_=o[:])

            # ---------------- Phase D: attention ----------------------------
            nkt = tb // P  # key tiles
            with tc.tile_pool(name="ath", bufs=2) as hp, \
                 tc.tile_pool(name="atex", bufs=4) as exp_p, \
                 tc.tile_pool(name="atev", bufs=4) as evp, \
                 tc.tile_pool(name="atsc", bufs=3, space="PSUM") as scp, \
                 tc.tile_pool(name="atpv", bufs=2, space="PSUM") as pvp, \
                 tc.tile_pool(name="atden", bufs=2, space="PSUM") as denp:
                for h in range(NH):
                    kn_h = hp.tile([P, tb], F32R, tag="knh", name=f"knh{h}")
                    nc.sync.dma_start(out=kn_h[:], in_=knope_d[:, h, :])
                    kr_h = hp.tile([DR, tb], F32R, tag="krh", name=f"krh{h}")
                    nc.sync.dma_start(
                        out=kr_h[:],
                        in_=krope_d[64 * (h % 2):64 * (h % 2) + 64, h // 2, :])
                    v_h = hp.tile([P, nkt, DV], F32R, tag="vh", name=f"vh{h}")
                    nc.sync.dma_start(out=v_h[:],
                                      in_=v_d[:, :, h * DV:(h + 1) * DV])
                    qn_h = hp.tile([P, tb], F32R, tag="qnh", name=f"qnh{h}")
                    nc.sync.dma_start(out=qn_h[:], in_=qnope_d[:, h, :])
                    qr_h = hp.tile([DR, tb], F32R, tag="qrh", name=f"qrh{h}")
                    nc.sync.dma_start(
                        out=qr_h[:],
                        in_=qrope_d[64 * (h % 2):64 * (h % 2) + 64, h // 2, :])
                    for qt in range(ntt):
                        qsl = slice(qt * TT, (qt + 1) * TT)
                        pv_ps = pvp.tile([P, TT], F32, tag="pv", name=f"pv{h}_{qt}")
                        den_ps = denp.tile([P, TT], F32, tag="den",
                                           name=f"den{h}_{qt}")
                        for kt in range(nkt):
                            sc_ps = scp.tile([P, TT], F32, tag="sc",
                                             name=f"sc{h}_{qt}_{kt}")
                            nc.tensor.matmul(sc_ps[:],
                                             kn_h[:, kt * P:(kt + 1) * P],
                                             qn_h[:, qsl],
                                             start=True, stop=False)
                            nc.tensor.matmul(sc_ps[:],
                                             kr_h[:, kt * P:(kt + 1) * P],
                                             qr_h[:, qsl],
                                             start=False, stop=True)
                            ex = exp_p.tile([P, TT], F32R, tag="ex",
                                            name=f"ex{h}_{qt}_{kt}")
                            nc.scalar.activation(ex[:], sc_ps[:], EXP_FN,
                                                 scale=SCALE)
                            nc.tensor.matmul(pv_ps[:], v_h[:, kt, :], ex[:],
                                             start=(kt == 0), stop=(kt == nkt - 1))
                            nc.tensor.matmul(den_ps[:], ones_r[:], ex[:],
                                             start=(kt == 0), stop=(kt == nkt - 1))
                        recip = evp.tile([P, TT], F32, tag="ev", name="recip")
                        nc.vector.reciprocal(recip[:], den_ps[:])
                        ao = evp.tile([P, TT], F32R, tag="ev", name="ao")
                        nc.vector.tensor_mul(out=ao[:], in0=pv_ps[:], in1=recip[:])
                        nc.sync.dma_start(out=attn_d[:, h, qsl], in_=ao[:])

            # ---------------- Phase E: o_proj (partial) ---------------------
            with tc.tile_pool(name="oww", bufs=1) as owp, \
                 tc.tile_pool(name="oin", bufs=3) as inp, \
                 tc.tile_pool(name="oev", bufs=3) as evp, \
                 tc.tile_pool(name="ops", bufs=4, space="PSUM") as ops:
                oww = owp.tile([P, NH * DV // P, HID], F32R)
                nc.sync.dma_start(out=oww[:], in_=ow_ap[:, :, :])
                for t8 in range(ntc):
                    at = inp.tile([P, NH, P], F32R, tag="at", name=f"at{t8}")
                    nc.sync.dma_start(out=at[:],
                                      in_=attn_d[:, :, t8 * P:(t8 + 1) * P])
                    for n in range(HID // TT):
                        ps = ops.tile([P, TT], F32, tag="ops", name=f"o{t8}_{n}")
                        for k in range(NH * DV // P):
                            nc.tensor.matmul(ps[:], at[:, k, :],
                                             oww[:, k, n * TT:(n + 1) * TT],
                                             start=(k == 0),
                                             stop=(k == NH * DV // P - 1))
                        o = evp.tile([P, TT], F32, tag="ev")
                        nc.vector.tensor_copy(out=o[:], in_=ps[:])
                        nc.sync.dma_start(
                            out=out_part[t8 * P:(t8 + 1) * P,
                                         n * TT:(n + 1) * TT],
                            in_=o[:])

    nc.compile()
    return nc


# ---------------------------------------------------------------------------
# host-side packing
# ---------------------------------------------------------------------------

def _rope_tables():
    inv_freq = 1.0 / (10000.0 ** (np.arange(0, DR, 2, dtype=np.float32) / DR))
    t = np.arange(MAXP, dtype=np.float32)
    freqs = np.outer(t, inv_freq)
    emb = np.concatenate([freqs, freqs], axis=-1)
    return np.cos(emb).astype(np.float32), np.sin(emb).astype(np.float32)


def core_weights(g, q_a_w, q_a_ln_w, q_b_w, kv_a_w, kv_a_ln_w, kv_b_w,
                 k_rope_w, o_w):
    """Pack the weight set for head-group g (heads g*8 .. g*8+8)."""
    heads = range(g * NH, (g + 1) * NH)
    qb_eff = (q_b_w * q_a_ln_w[None, :]).astype(np.float32)
    kvb_eff = (kv_b_w * kv_a_ln_w[None, :]).astype(np.float32)

    nope_rows = np.concatenate(
        [np.arange(h * (DN + DR), h * (DN + DR) + DN) for h in heads])
    lo_rows = np.concatenate(
        [np.arange(h * (DN + DR) + DN, h * (DN + DR) + DN + 32) for h in heads])
    hi_rows = np.concatenate(
        [np.arange(h * (DN + DR) + DN + 32, h * (DN + DR) + DN + 64)
         for h in heads])
    qb_rows = np.concatenate([nope_rows, lo_rows, hi_rows])

    k_rows = np.concatenate(
        [np.arange(h * (DN + DV), h * (DN + DV) + DN) for h in heads])
    v_rows = np.concatenate(
        [np.arange(h * (DN + DV) + DN, (h + 1) * (DN + DV)) for h in heads])

    kr_lo = np.concatenate([np.arange(h * DR, h * DR + 32) for h in heads])
    kr_hi = np.concatenate([np.arange(h * DR + 32, (h + 1) * DR) for h in heads])
    kr_rows = np.concatenate([kr_lo, kr_hi])

    o_cols = np.concatenate([np.arange(h * DV, (h + 1) * DV) for h in heads])

    c = np.ascontiguousarray
    return {
        "qa_wT": c(q_a_w.T),
        "kva_wT": c(kv_a_w.T),
        "kr_wT": c(k_rope_w[kr_rows].T),
        "qb_wT": c(qb_eff[qb_rows].T),
        "kvbk_wT": c(kvb_eff[k_rows].T),
        "kvbv_wT": c(kvb_eff[v_rows].T),
        "o_wT": c(o_w[:, o_cols].T),
    }


def core_inputs(c, hidden_states, position_ids, weight_sets, tb=S):
    b, g = c // 4, c % 4
    cos_tab, sin_tab = _rope_tables()
    pos = np.asarray(position_ids[b][:tb])
    cos_b = cos_tab[pos]  # [tb, DR]
    sin_b = sin_tab[pos]
    cos_rep = np.ascontiguousarray(np.tile(cos_b[:, :32].T, (4, 1)))
    sin_rep = np.ascontiguousarray(np.tile(sin_b[:, :32].T, (4, 1)))
    x = np.asarray(hidden_states[b][:tb], dtype=np.float32)
    im = {"xT": np.ascontiguousarray(x.T), "cos_rep": cos_rep,
          "sin_rep": sin_rep}
    im.update(weight_sets[g])
    return im


_CACHE = {}


def _get_nc(tb=S):
    if tb not in _CACHE:
        _CACHE[tb] = build_nc(tb)
    return _CACHE[tb]


def kernel(hidden_states, position_ids, q_a_w, q_a_ln_w, q_b_w,
           kv_a_w, kv_a_ln_w, kv_b_w, k_rope_w, o_w):
    hidden_states = np.asarray(hidden_states, dtype=np.float32)
    weight_sets = [
        core_weights(g, np.asarray(q_a_w, np.float32),
                     np.asarray(q_a_ln_w, np.float32),
                     np.asarray(q_b_w, np.float32),
                     np.asarray(kv_a_w, np.float32),
                     np.asarray(kv_a_ln_w, np.float32),
                     np.asarray(kv_b_w, np.float32),
                     np.asarray(k_rope_w, np.float32),
                     np.asarray(o_w, np.float32))
        for g in range(4)
    ]
    in_maps = [core_inputs(c, hidden_states, position_ids, weight_sets)
               for c in range(NCORES)]
    nc = _get_nc()
    res = run_bass_kernel_spmd(nc, in_maps, core_ids=list(range(NCORES)))
    out = np.zeros((B, S, HID), dtype=np.float64)
    for c in range(NCORES):
        out[c // 4] += res.results[c]["out_part"]
    return out.astype(np.float32)



# revision 5
# speedup vs baseline: 1.2898x; 1.2898x over previous
"""DeepSeekV3 MLA attention kernel for Trainium2 (8 NeuronCores, Bass/Tile).

Sharding: core c -> batch b = c // 4, head-group g = c % 4 (8 of 32 heads),
token-quarter q = c % 4.  The low-rank a-projections (q_a, kv_a) are
sequence-parallel: each core computes + rms-normalizes the latents for its
own quarter of the 2048 tokens, then an in-group AllGather (cores 0-3 /
4-7) replicates the full normalized latents to every core.  Everything
else (k_rope, q_b, kv_b, attention, o_proj) is head-sharded as before;
each core emits a partial o_proj output [2048, 4096] and the host sums
the 4 partials per batch.

Layouts (feature-major, [128, chunks, tokens]):
  - x is fed transposed (xT [4096, 2048]); all matmuls contract over the
    partition dim with N = 512 token tiles (one PSUM bank, full fp32r rate).
  - RoPE halves are packed [4*lo(128) | 4*hi(128)] per 4 heads so the rotate
    is partition-aligned full-lane DVE work; a DMA rearrange then stores the
    per-head-contiguous [64] blocks that attention contracts over (K=64).
  - Softmax skips the max-subtraction (scores are O(5) here, exp is safe in
    fp32); denominators come from an all-ones matmul accumulated alongside
    the PV matmul.
  - o_proj runs n-outer with the attention output SBUF-resident (written in
    place by phase D) and o_w streamed once.

All matmul operands are float32r (TF32-like); PSUM accumulation is fp32.

build_nc(sim_local=True) replaces the collective with local computation of
all four quarters so CoreSim can check numerics single-core.
"""

import math

import numpy as np

try:
    import concourse.bacc as bacc  # noqa: F401
except ImportError:
    import sys

    for _p in ("/root/.axon_site/_ro/trn_rl_repo", "/opt/trn_rl_repo"):
        if _p not in sys.path:
            sys.path.insert(0, _p)

import concourse.bacc as bacc
import concourse.mybir as mybir
import concourse.tile as tile
from concourse.bass_utils import run_bass_kernel_spmd

# model dims
H, DN, DR, DV = 32, 128, 64, 128
HID, QR, KVR = 4096, 1536, 512
EPS, MAXP = 1e-6, 4096
B, S = 2, 2048
P = 128
TT = 512  # token tile (matmul moving dim)
NH = 8  # heads per core
NCORES = 8
SCALE = 1.0 / math.sqrt(DN + DR)
HIDC = HID // P  # 32
QRC = QR // P  # 12
KVRC = KVR // P  # 4
QKC = QRC + KVRC  # 16 gathered latent chunks

F32 = mybir.dt.float32
F32R = mybir.dt.float32r

EXP_FN = mybir.ActivationFunctionType.Exp
SQRT_FN = mybir.ActivationFunctionType.Sqrt


def build_nc(tb=S, sim_local=False):
    """Build the per-core Bass program (same program on all 8 cores)."""
    ntt = tb // TT
    ntc = tb // P  # token chunks
    tq = tb // 4  # quarter size (seq-parallel slice)
    nc = bacc.Bacc("TRN2", target_bir_lowering=False, debug=False)

    xT = nc.dram_tensor("xT", [HID, tb], F32R, kind="ExternalInput")
    if not sim_local:
        xqT = nc.dram_tensor("xqT", [HID, tq], F32R, kind="ExternalInput")
    qa_wT = nc.dram_tensor("qa_wT", [HID, QR], F32R, kind="ExternalInput")
    kva_wT = nc.dram_tensor("kva_wT", [HID, KVR], F32R, kind="ExternalInput")
    kr_wT = nc.dram_tensor("kr_wT", [HID, NH * DR], F32R, kind="ExternalInput")
    qb_wT = nc.dram_tensor("qb_wT", [QR, NH * (DN + DR)], F32R, kind="ExternalInput")
    kvbk_wT = nc.dram_tensor("kvbk_wT", [KVR, NH * DN], F32R, kind="ExternalInput")
    kvbv_wT = nc.dram_tensor("kvbv_wT", [KVR, NH * DV], F32R, kind="ExternalInput")
    o_wT = nc.dram_tensor("o_wT", [NH * DV, HID], F32R, kind="ExternalInput")
    cos_in = nc.dram_tensor("cos_rep", [P, tb], F32, kind="ExternalInput")
    sin_in = nc.dram_tensor("sin_rep", [P, tb], F32, kind="ExternalInput")
    out_part = nc.dram_tensor("out_part", [tb, HID], F32, kind="ExternalOutput")

    x_ap = xT[:, :].rearrange("(c p) t -> p c t", p=P)
    if not sim_local:
        xq_ap = xqT[:, :].rearrange("(c p) t -> p c t", p=P)
    qa_ap = qa_wT[:, :].rearrange("(c p) m -> p c m", p=P)
    kva_ap = kva_wT[:, :].rearrange("(c p) m -> p c m", p=P)
    kr_ap = kr_wT[:, :].rearrange("(c p) m -> p c m", p=P)
    qb_ap = qb_wT[:, :].rearrange("(c p) m -> p c m", p=P)
    kvbk_ap = kvbk_wT[:, :].rearrange("(c p) m -> p c m", p=P)
    kvbv_ap = kvbv_wT[:, :].rearrange("(c p) m -> p c m", p=P)
    ow_ap = o_wT[:, :].rearrange("(c p) m -> p c m", p=P)

    with tile.TileContext(nc) as tc:
        with tc.tile_pool(name="const", bufs=1) as constp, \
             tc.tile_pool(name="dram", bufs=1, space="DRAM") as dram:
            ones_f = constp.tile([P, P], F32)
            nc.any.memset(ones_f[:], 1.0)
            ones_r = constp.tile([P, P], F32R)
            nc.vector.tensor_copy(out=ones_r[:], in_=ones_f[:])
            eps_sb = constp.tile([P, 1], F32)
            nc.any.memset(eps_sb[:], EPS)
            cos_sb = constp.tile([P, tb], F32)
            sin_sb = constp.tile([P, tb], F32)
            nc.sync.dma_start(out=cos_sb[:], in_=cos_in[:, :])
            nc.sync.dma_start(out=sin_sb[:], in_=sin_in[:, :])

            # gathered normalized latents: slot s = quarter s of the batch
            qkv_all = dram.tile([4, P, QKC, tq], F32R)
            if not sim_local:
                qkv_in = dram.tile([P, QKC, tq], F32R)
            qnope_d = dram.tile([P, NH, tb], F32R)
            qrope_d = dram.tile([P, NH * DR // P, tb], F32R)
            knope_d = dram.tile([P, NH, tb], F32R)
            krope_d = dram.tile([P, NH * DR // P, tb], F32R)
            v_d = dram.tile([P, ntc, NH * DV], F32R)

            def rope_evict(lo_src, hi_src, tsl, pool, tag):
                """lo/hi chunk pair [P, TT] (4 heads x 32 rows) -> rotate."""
                t1 = pool.tile([P, TT], F32, tag=tag, name="rt1")
                t2 = pool.tile([P, TT], F32, tag=tag, name="rt2")
                nc.vector.tensor_mul(out=t1[:], in0=lo_src[:], in1=cos_sb[:, tsl])
                nc.vector.tensor_mul(out=t2[:], in0=hi_src[:], in1=sin_sb[:, tsl])
                lo_o = pool.tile([P, TT], F32R, tag=tag, name="rlo")
                nc.vector.tensor_sub(out=lo_o[:], in0=t1[:], in1=t2[:])
                t3 = pool.tile([P, TT], F32, tag=tag, name="rt3")
                t4 = pool.tile([P, TT], F32, tag=tag, name="rt4")
                nc.vector.tensor_mul(out=t3[:], in0=hi_src[:], in1=cos_sb[:, tsl])
                nc.vector.tensor_mul(out=t4[:], in0=lo_src[:], in1=sin_sb[:, tsl])
                hi_o = pool.tile([P, TT], F32R, tag=tag, name="rhi")
                nc.vector.tensor_add(out=hi_o[:], in0=t3[:], in1=t4[:])
                return lo_o, hi_o

            def rope_store(lo_o, hi_o, ci, dst_d, tsl):
                for hh in range(4):
                    h = ci * 4 + hh
                    dc, dp = h // 2, 64 * (h % 2)
                    nc.sync.dma_start(
                        out=dst_d[dp:dp + 32, dc, tsl],
                        in_=lo_o[32 * hh:32 * hh + 32, :])
                    nc.sync.dma_start(
                        out=dst_d[dp + 32:dp + 64, dc, tsl],
                        in_=hi_o[32 * hh:32 * hh + 32, :])

            # ---------------- Phase A1: seq-parallel a-projections ----------
            # groups of (kind, m0, m1) output chunks over HID contraction;
            # chunk index in the gathered latent: q m -> m, kv m -> 12 + m.
            agroups = [("q", 0, 4), ("q", 4, 8), ("q", 8, 12), ("kv", 0, 4)]
            asrc = {"q": qa_ap, "kv": kva_ap}

            def aproj(xsrc, dst_slot):
                """a-projections for one token quarter; xsrc: [P, HIDC, tq]
                chunk AP; writes normalized latents into dst_slot AP
                ([P, QKC, tq])."""
                with tc.tile_pool(name="axq", bufs=HIDC + 1) as xqp, \
                     tc.tile_pool(name="apw", bufs=4) as wpool, \
                     tc.tile_pool(name="araw", bufs=QKC + 1) as rawp, \
                     tc.tile_pool(name="aev", bufs=8) as evp, \
                     tc.tile_pool(name="aacc", bufs=5, space="PSUM") as accp, \
                     tc.tile_pool(name="astat", bufs=2, space="PSUM") as statp:
                    xq = []
                    for k in range(HIDC):
                        xt = xqp.tile([P, tq], F32R, tag="axq", name=f"axq{k}")
                        nc.sync.dma_start(out=xt[:], in_=xsrc[:, k, :])
                        xq.append(xt)
                    statq = statp.tile([P, tq], F32, tag="stat", name="statq")
                    statkv = statp.tile([P, tq], F32, tag="stat", name="statkv")
                    raws = {}
                    for gi, (kind, m0, m1) in enumerate(agroups):
                        src = asrc[kind]
                        nm = m1 - m0
                        accs = [accp.tile([P, tq], F32, tag="acc",
                                          name=f"acc{gi}_{m}")
                                for m in range(m0, m1)]
                        for k in range(HIDC):
                            wt = wpool.tile([P, nm * P], F32R, tag="apw",
                                            name=f"apw{gi}_{k}")
                            nc.sync.dma_start(out=wt[:],
                                              in_=src[:, k, m0 * P:m1 * P])
                            for mi in range(nm):
                                nc.tensor.matmul(
                                    accs[mi][:], wt[:, mi * P:(mi + 1) * P],
                                    xq[k][:], start=(k == 0),
                                    stop=(k == HIDC - 1))
                        stat = statq if kind == "q" else statkv
                        nmax = QRC if kind == "q" else KVRC
                        for mi, m in enumerate(range(m0, m1)):
                            raw = rawp.tile([P, tq], F32R, tag="araw",
                                            name=f"raw{gi}_{m}")
                            nc.vector.tensor_copy(out=raw[:], in_=accs[mi][:])
                            raws[(kind, m)] = raw
                            sq = evp.tile([P, tq], F32R, tag="aev")
                            nc.vector.tensor_mul(out=sq[:], in0=raw[:],
                                                 in1=raw[:])
                            nc.tensor.matmul(stat[:], ones_r[:], sq[:],
                                             start=(m == 0),
                                             stop=(m == nmax - 1))
                    for kind, nrank, stat in (("q", QR, statq),
                                              ("kv", KVR, statkv)):
                        sdev = evp.tile([P, tq], F32, tag="aev")
                        nc.scalar.activation(sdev[:], stat[:], SQRT_FN,
                                             bias=eps_sb[:], scale=1.0 / nrank)
                        rstd = evp.tile([P, tq], F32, tag="aev",
                                        name=f"rstd_{kind}")
                        nc.vector.reciprocal(rstd[:], sdev[:])
                        nmax = QRC if kind == "q" else KVRC
                        coff = 0 if kind == "q" else QRC
                        for m in range(nmax):
                            o = evp.tile([P, tq], F32R, tag="aev")
                            nc.vector.tensor_mul(out=o[:],
                                                 in0=raws[(kind, m)][:],
                                                 in1=rstd[:])
                            nc.sync.dma_start(out=dst_slot[:, coff + m, :],
                                              in_=o[:])

            if sim_local:
                for s in range(4):
                    aproj(x_ap[:, :, s * tq:(s + 1) * tq], qkv_all[s, :, :, :])
            else:
                aproj(xq_ap[:, :, :], qkv_in[:, :, :])
                nc.gpsimd.collective_compute(
                    "AllGather",
                    mybir.AluOpType.bypass,
                    replica_groups=[[0, 1, 2, 3], [4, 5, 6, 7]],
                    ins=[qkv_in.opt()],
                    outs=[qkv_all.opt()],
                )

            # ---------------- Phase A2: k_rope over all tokens --------------
            with tc.tile_pool(name="krw", bufs=1) as krwp, \
                 tc.tile_pool(name="krx", bufs=4) as xpool, \
                 tc.tile_pool(name="krev", bufs=10) as evp, \
                 tc.tile_pool(name="kracc", bufs=4, space="PSUM") as accp:
                krw = krwp.tile([P, HIDC, NH * DR], F32R)
                nc.sync.dma_start(out=krw[:], in_=kr_ap[:, :, :])
                for t in range(ntt):
                    tsl = slice(t * TT, (t + 1) * TT)
                    accs = [accp.tile([P, TT], F32, tag="acc",
                                      name=f"kracc{t}_{m}")
                            for m in range(4)]
                    for k in range(HIDC):
                        xt = xpool.tile([P, TT], F32R, tag="krx",
                                        name=f"krx{t}_{k}")
                        nc.sync.dma_start(out=xt[:], in_=x_ap[:, k, tsl])
                        for mi in range(4):
                            nc.tensor.matmul(
                                accs[mi][:], krw[:, k, mi * P:(mi + 1) * P],
                                xt[:], start=(k == 0), stop=(k == HIDC - 1))
                    for ci in range(2):
                        lo_o, hi_o = rope_evict(accs[ci], accs[2 + ci], tsl,
                                                evp, "krev")
                        rope_store(lo_o, hi_o, ci, krope_d, tsl)

            # ---------------- Phase B: q_b + q rope -------------------------
            nspt = TT // tq if TT > tq else 1  # gathered slots per token tile
            with tc.tile_pool(name="qbw", bufs=1) as qbwp, \
                 tc.tile_pool(name="qbn", bufs=2) as qnp, \
                 tc.tile_pool(name="qbev", bufs=10) as evp, \
                 tc.tile_pool(name="qbps", bufs=4, space="PSUM") as qbps:
                qbw = qbwp.tile([P, QRC, NH * (DN + DR)], F32R)
                nc.sync.dma_start(out=qbw[:], in_=qb_ap[:, :, :])
                for t in range(ntt):
                    tsl = slice(t * TT, (t + 1) * TT)
                    qn = qnp.tile([P, QRC, TT], F32R, tag="qn", name=f"qn{t}")
                    for si in range(nspt):
                        s = (t * TT) // tq + si
                        nc.sync.dma_start(
                            out=qn[:, :, si * tq:(si + 1) * tq]
                            if nspt > 1 else qn[:, :, :],
                            in_=qkv_all[s, :, 0:QRC, :])
                    rope_ps = {}
                    for mg in range(3):
                        pss = []
                        for m in range(mg * 4, mg * 4 + 4):
                            ps = qbps.tile([P, TT], F32, tag="qbps",
                                           name=f"qb{t}_{m}")
                            for k in range(QRC):
                                nc.tensor.matmul(
                                    ps[:], qbw[:, k, m * P:(m + 1) * P],
                                    qn[:, k, :], start=(k == 0),
                                    stop=(k == QRC - 1))
                            pss.append(ps)
                        if mg < 2:
                            for mi, ps in enumerate(pss):
                                o = evp.tile([P, TT], F32R, tag="qbev")
                                nc.vector.tensor_copy(out=o[:], in_=ps[:])
                                nc.sync.dma_start(
                                    out=qnope_d[:, mg * 4 + mi, tsl], in_=o[:])
                        else:
                            for mi, ps in enumerate(pss):
                                rope_ps[mi] = ps
                    for ci in range(2):
                        lo_o, hi_o = rope_evict(rope_ps[ci], rope_ps[2 + ci],
                                                tsl, evp, "qbev")
                        rope_store(lo_o, hi_o, ci, qrope_d, tsl)

            # ---------------- Phase C: kv_b (k_nope + v) --------------------
            with tc.tile_pool(name="kvw", bufs=1) as kvwp, \
                 tc.tile_pool(name="kvn", bufs=2) as kvnp, \
                 tc.tile_pool(name="kvev", bufs=6) as evp, \
                 tc.tile_pool(name="kvps", bufs=4, space="PSUM") as kvps:
                kbw = kvwp.tile([P, KVRC, NH * DN], F32R)
                vbw = kvwp.tile([P, KVRC, NH * DV], F32R)
                nc.sync.dma_start(out=kbw[:], in_=kvbk_ap[:, :, :])
                nc.sync.dma_start(out=vbw[:], in_=kvbv_ap[:, :, :])
                for t in range(ntt):
                    tsl = slice(t * TT, (t + 1) * TT)
                    kvn = kvnp.tile([P, KVRC, TT], F32R, tag="kvn",
                                    name=f"kvn{t}")
                    for si in range(nspt):
                        s = (t * TT) // tq + si
                        nc.sync.dma_start(
                            out=kvn[:, :, si * tq:(si + 1) * tq]
                            if nspt > 1 else kvn[:, :, :],
                            in_=qkv_all[s, :, QRC:QKC, :])
                    for m in range(NH):
                        ps = kvps.tile([P, TT], F32, tag="kps",
                                       name=f"kb{t}_{m}")
                        for k in range(KVRC):
                            nc.tensor.matmul(ps[:], kbw[:, k, m * P:(m + 1) * P],
                                             kvn[:, k, :], start=(k == 0),
                                             stop=(k == KVRC - 1))
                        o = evp.tile([P, TT], F32R, tag="kvev")
                        nc.vector.tensor_copy(out=o[:], in_=ps[:])
                        nc.sync.dma_start(out=knope_d[:, m, tsl], in_=o[:])
                    for tc8 in range(TT // P):
                        for vc in range(NH * DV // TT):
                            ps = kvps.tile([P, TT], F32, tag="vps",
                                           name=f"v{t}_{tc8}_{vc}")
                            for k in range(KVRC):
                                nc.tensor.matmul(
                                    ps[:],
                                    kvn[:, k, tc8 * P:(tc8 + 1) * P],
                                    vbw[:, k, vc * TT:(vc + 1) * TT],
                                    start=(k == 0), stop=(k == KVRC - 1))
                            o = evp.tile([P, TT], F32R, tag="kvev")
                            nc.vector.tensor_copy(out=o[:], in_=ps[:])
                            nc.sync.dma_start(
                                out=v_d[:, t * (TT // P) + tc8,
                                        vc * TT:(vc + 1) * TT],
                                in_=o[:])

            # ---------------- Phase D: attention ----------------------------
            # attention output accumulates in SBUF (attn_sb) for phase E.
            nkt = tb // P  # key tiles
            with tc.tile_pool(name="aosb", bufs=1) as aop:
                attn_sb = aop.tile([P, NH, tb], F32R)
                with tc.tile_pool(name="ath", bufs=2) as hp, \
                     tc.tile_pool(name="atex", bufs=4) as exp_p, \
                     tc.tile_pool(name="atev", bufs=4) as evp, \
                     tc.tile_pool(name="atsc", bufs=3, space="PSUM") as scp, \
                     tc.tile_pool(name="atpv", bufs=2, space="PSUM") as pvp, \
                     tc.tile_pool(name="atden", bufs=2, space="PSUM") as denp:
                    for h in range(NH):
                        kn_h = hp.tile([P, tb], F32R, tag="knh", name=f"knh{h}")
                        nc.sync.dma_start(out=kn_h[:], in_=knope_d[:, h, :])
                        kr_h = hp.tile([DR, tb], F32R, tag="krh", name=f"krh{h}")
                        nc.sync.dma_start(
                            out=kr_h[:],
                            in_=krope_d[64 * (h % 2):64 * (h % 2) + 64,
                                        h // 2, :])
                        v_h = hp.tile([P, nkt, DV], F32R, tag="vh", name=f"vh{h}")
                        nc.sync.dma_start(out=v_h[:],
                                          in_=v_d[:, :, h * DV:(h + 1) * DV])
                        qn_h = hp.tile([P, tb], F32R, tag="qnh", name=f"qnh{h}")
                        nc.sync.dma_start(out=qn_h[:], in_=qnope_d[:, h, :])
                        qr_h = hp.tile([DR, tb], F32R, tag="qrh", name=f"qrh{h}")
                        nc.sync.dma_start(
                            out=qr_h[:],
                            in_=qrope_d[64 * (h % 2):64 * (h % 2) + 64,
                                        h // 2, :])
                        for qt in range(ntt):
                            qsl = slice(qt * TT, (qt + 1) * TT)
                            pv_ps = pvp.tile([P, TT], F32, tag="pv",
                                             name=f"pv{h}_{qt}")
                            den_ps = denp.tile([P, TT], F32, tag="den",
                                               name=f"den{h}_{qt}")
                            for kt in range(nkt):
                                sc_ps = scp.tile([P, TT], F32, tag="sc",
                                                 name=f"sc{h}_{qt}_{kt}")
                                nc.tensor.matmul(sc_ps[:],
                                                 kn_h[:, kt * P:(kt + 1) * P],
                                                 qn_h[:, qsl],
                                                 start=True, stop=False)
                                nc.tensor.matmul(sc_ps[:],
                                                 kr_h[:, kt * P:(kt + 1) * P],
                                                 qr_h[:, qsl],
                                                 start=False, stop=True)
                                ex = exp_p.tile([P, TT], F32R, tag="ex",
                                                name=f"ex{h}_{qt}_{kt}")
                                nc.scalar.activation(ex[:], sc_ps[:], EXP_FN,
                                                     scale=SCALE)
                                nc.tensor.matmul(pv_ps[:], v_h[:, kt, :], ex[:],
                                                 start=(kt == 0),
                                                 stop=(kt == nkt - 1))
                                nc.tensor.matmul(den_ps[:], ones_r[:], ex[:],
                                                 start=(kt == 0),
                                                 stop=(kt == nkt - 1))
                            recip = evp.tile([P, TT], F32, tag="atev",
                                             name="recip")
                            nc.vector.reciprocal(recip[:], den_ps[:])
                            nc.vector.tensor_mul(out=attn_sb[:, h, qsl],
                                                 in0=pv_ps[:], in1=recip[:])

                # ------------ Phase E: o_proj (partial), n-outer ------------
                with tc.tile_pool(name="oww", bufs=3) as owp, \
                     tc.tile_pool(name="oev", bufs=4) as evp2, \
                     tc.tile_pool(name="ops", bufs=4, space="PSUM") as ops:
                    for n in range(HID // TT):
                        oww = owp.tile([P, NH * DV // P, TT], F32R, tag="oww",
                                       name=f"oww{n}")
                        nc.sync.dma_start(
                            out=oww[:], in_=ow_ap[:, :, n * TT:(n + 1) * TT])
                        for t8 in range(ntc):
                            ps = ops.tile([P, TT], F32, tag="ops",
                                          name=f"o{n}_{t8}")
                            for k in range(NH * DV // P):
                                nc.tensor.matmul(
                                    ps[:],
                                    attn_sb[:, k, t8 * P:(t8 + 1) * P],
                                    oww[:, k, :],
                                    start=(k == 0),
                                    stop=(k == NH * DV // P - 1))
                            o = evp2.tile([P, TT], F32, tag="oev")
                            nc.vector.tensor_copy(out=o[:], in_=ps[:])
                            nc.sync.dma_start(
                                out=out_part[t8 * P:(t8 + 1) * P,
                                             n * TT:(n + 1) * TT],
                                in_=o[:])

    nc.compile()
    return nc


# ---------------------------------------------------------------------------
# host-side packing
# ---------------------------------------------------------------------------

def _rope_tables():
    inv_freq = 1.0 / (10000.0 ** (np.arange(0, DR, 2, dtype=np.float32) / DR))
    t = np.arange(MAXP, dtype=np.float32)
    freqs = np.outer(t, inv_freq)
    emb = np.concatenate([freqs, freqs], axis=-1)
    return np.cos(emb).astype(np.float32), np.sin(emb).astype(np.float32)


def core_weights(g, q_a_w, q_a_ln_w, q_b_w, kv_a_w, kv_a_ln_w, kv_b_w,
                 k_rope_w, o_w):
    """Pack the weight set for head-group g (heads g*8 .. g*8+8)."""
    heads = range(g * NH, (g + 1) * NH)
    qb_eff = (q_b_w * q_a_ln_w[None, :]).astype(np.float32)
    kvb_eff = (kv_b_w * kv_a_ln_w[None, :]).astype(np.float32)

    nope_rows = np.concatenate(
        [np.arange(h * (DN + DR), h * (DN + DR) + DN) for h in heads])
    lo_rows = np.concatenate(
        [np.arange(h * (DN + DR) + DN, h * (DN + DR) + DN + 32) for h in heads])
    hi_rows = np.concatenate(
        [np.arange(h * (DN + DR) + DN + 32, h * (DN + DR) + DN + 64)
         for h in heads])
    qb_rows = np.concatenate([nope_rows, lo_rows, hi_rows])

    k_rows = np.concatenate(
        [np.arange(h * (DN + DV), h * (DN + DV) + DN) for h in heads])
    v_rows = np.concatenate(
        [np.arange(h * (DN + DV) + DN, (h + 1) * (DN + DV)) for h in heads])

    kr_lo = np.concatenate([np.arange(h * DR, h * DR + 32) for h in heads])
    kr_hi = np.concatenate([np.arange(h * DR + 32, (h + 1) * DR) for h in heads])
    kr_rows = np.concatenate([kr_lo, kr_hi])

    o_cols = np.concatenate([np.arange(h * DV, (h + 1) * DV) for h in heads])

    c = np.ascontiguousarray
    return {
        "qa_wT": c(q_a_w.T),
        "kva_wT": c(kv_a_w.T),
        "kr_wT": c(k_rope_w[kr_rows].T),
        "qb_wT": c(qb_eff[qb_rows].T),
        "kvbk_wT": c(kvb_eff[k_rows].T),
        "kvbv_wT": c(kvb_eff[v_rows].T),
        "o_wT": c(o_w[:, o_cols].T),
    }


def core_inputs(c, hidden_states, position_ids, weight_sets, tb=S,
                sim_local=False):
    b, g = c // 4, c % 4
    tq = tb // 4
    cos_tab, sin_tab = _rope_tables()
    pos = np.asarray(position_ids[b][:tb])
    cos_b = cos_tab[pos]  # [tb, DR]
    sin_b = sin_tab[pos]
    cos_rep = np.ascontiguousarray(np.tile(cos_b[:, :32].T, (4, 1)))
    sin_rep = np.ascontiguousarray(np.tile(sin_b[:, :32].T, (4, 1)))
    x = np.asarray(hidden_states[b][:tb], dtype=np.float32)
    xT = np.ascontiguousarray(x.T)
    im = {"xT": xT, "cos_rep": cos_rep, "sin_rep": sin_rep}
    if not sim_local:
        im["xqT"] = np.ascontiguousarray(xT[:, g * tq:(g + 1) * tq])
    im.update(weight_sets[g])
    return im


_CACHE = {}


def _get_nc(tb=S):
    if tb not in _CACHE:
        _CACHE[tb] = build_nc(tb)
    return _CACHE[tb]


def kernel(hidden_states, position_ids, q_a_w, q_a_ln_w, q_b_w,
           kv_a_w, kv_a_ln_w, kv_b_w, k_rope_w, o_w):
    hidden_states = np.asarray(hidden_states, dtype=np.float32)
    weight_sets = [
        core_weights(g, np.asarray(q_a_w, np.float32),
                     np.asarray(q_a_ln_w, np.float32),
                     np.asarray(q_b_w, np.float32),
                     np.asarray(kv_a_w, np.float32),
                     np.asarray(kv_a_ln_w, np.float32),
                     np.asarray(kv_b_w, np.float32),
                     np.asarray(k_rope_w, np.float32),
                     np.asarray(o_w, np.float32))
        for g in range(4)
    ]
    in_maps = [core_inputs(c, hidden_states, position_ids, weight_sets)
               for c in range(NCORES)]
    nc = _get_nc()
    res = run_bass_kernel_spmd(nc, in_maps, core_ids=list(range(NCORES)))
    out = np.zeros((B, S, HID), dtype=np.float64)
    for c in range(NCORES):
        out[c // 4] += res.results[c]["out_part"]
    return out.astype(np.float32)


# revision 6
# speedup vs baseline: 1.7387x; 1.3480x over previous
"""DeepSeekV3 MLA attention kernel for Trainium2 (8 NeuronCores, Bass/Tile).

Sharding: core c -> batch b = c // 4, head-group g = c % 4 (8 of 32 heads),
token-quarter q = c % 4.  The low-rank a-projections (q_a, kv_a) are
sequence-parallel: each core computes + rms-normalizes the latents for its
own quarter of the 2048 tokens, then two in-group AllGathers (kv first so
kv_b unblocks early, then q) replicate the full normalized latents.
Everything else (k_rope, q_b, kv_b, attention, o_proj) is head-sharded;
each core emits a partial o_proj output [2048, 4096] and the host sums
the 4 partials per batch.

Everything flows in bf16 (PSUM accumulation stays fp32): measured end
numerics ~5e-3 rel-err vs the 2e-2 gate.  bf16 halves DMA traffic, the
collective payload, and SBUF footprints - which lets the q/k intermediates
(qnope/qrope/knope/krope) stay SBUF-resident from production (B/C/k_rope)
through attention; only V round-trips through DRAM.

Layouts are feature-major [128, chunk, token]; matmuls contract over the
partition dim with 512-token moving tiles (one PSUM bank per accumulator).
RoPE halves are packed [4*lo(128) | 4*hi(128)] per 4 heads so the rotate is
partition-aligned full-lane DVE work; a DMA rearrange then stores
per-head-contiguous [64] blocks that attention contracts over.  Softmax
skips the max-subtraction (scores are O(5), exp is safe); denominators come
from an all-ones matmul accumulated alongside the PV matmul.  o_proj runs
n-outer with the attention output SBUF-resident and o_w streamed once.

build_nc(sim_local=True) replaces the collectives with local computation of
all four quarters so CoreSim can check numerics single-core.
"""

import math

import numpy as np

try:
    import concourse.bacc as bacc  # noqa: F401
except ImportError:
    import sys

    for _p in ("/root/.axon_site/_ro/trn_rl_repo", "/opt/trn_rl_repo"):
        if _p not in sys.path:
            sys.path.insert(0, _p)

import concourse.bacc as bacc
import concourse.mybir as mybir
import concourse.tile as tile
from concourse.bass_utils import run_bass_kernel_spmd

# model dims
H, DN, DR, DV = 32, 128, 64, 128
HID, QR, KVR = 4096, 1536, 512
EPS, MAXP = 1e-6, 4096
B, S = 2, 2048
P = 128
TT = 512  # token tile (matmul moving dim)
NH = 8  # heads per core
NCORES = 8
SCALE = 1.0 / math.sqrt(DN + DR)
HIDC = HID // P  # 32
QRC = QR // P  # 12
KVRC = KVR // P  # 4

F32 = mybir.dt.float32
BF16 = mybir.dt.bfloat16

EXP_FN = mybir.ActivationFunctionType.Exp
SQRT_FN = mybir.ActivationFunctionType.Sqrt

RGROUPS = [[0, 1, 2, 3], [4, 5, 6, 7]]


def build_nc(tb=S, sim_local=False):
    """Build the per-core Bass program (same program on all 8 cores)."""
    ntt = tb // TT
    ntc = tb // P  # token chunks
    tq = tb // 4  # quarter size (seq-parallel slice)
    nc = bacc.Bacc("TRN2", target_bir_lowering=False, debug=False)

    xT = nc.dram_tensor("xT", [HID, tb], BF16, kind="ExternalInput")
    if not sim_local:
        xqT = nc.dram_tensor("xqT", [HID, tq], BF16, kind="ExternalInput")
    qa_wT = nc.dram_tensor("qa_wT", [HID, QR], BF16, kind="ExternalInput")
    kva_wT = nc.dram_tensor("kva_wT", [HID, KVR], BF16, kind="ExternalInput")
    kr_wT = nc.dram_tensor("kr_wT", [HID, NH * DR], BF16, kind="ExternalInput")
    qb_wT = nc.dram_tensor("qb_wT", [QR, NH * (DN + DR)], BF16, kind="ExternalInput")
    kvbk_wT = nc.dram_tensor("kvbk_wT", [KVR, NH * DN], BF16, kind="ExternalInput")
    kvbv_wT = nc.dram_tensor("kvbv_wT", [KVR, NH * DV], BF16, kind="ExternalInput")
    o_wT = nc.dram_tensor("o_wT", [NH * DV, HID], BF16, kind="ExternalInput")
    cos_in = nc.dram_tensor("cos_rep", [P, tb], F32, kind="ExternalInput")
    sin_in = nc.dram_tensor("sin_rep", [P, tb], F32, kind="ExternalInput")
    out_part = nc.dram_tensor("out_part", [tb, HID], F32, kind="ExternalOutput")

    x_ap = xT[:, :].rearrange("(c p) t -> p c t", p=P)
    if not sim_local:
        xq_ap = xqT[:, :].rearrange("(c p) t -> p c t", p=P)
    qa_ap = qa_wT[:, :].rearrange("(c p) m -> p c m", p=P)
    kva_ap = kva_wT[:, :].rearrange("(c p) m -> p c m", p=P)
    kr_ap = kr_wT[:, :].rearrange("(c p) m -> p c m", p=P)
    qb_ap = qb_wT[:, :].rearrange("(c p) m -> p c m", p=P)
    kvbk_ap = kvbk_wT[:, :].rearrange("(c p) m -> p c m", p=P)
    kvbv_ap = kvbv_wT[:, :].rearrange("(c p) m -> p c m", p=P)
    ow_ap = o_wT[:, :].rearrange("(c p) m -> p c m", p=P)

    with tile.TileContext(nc) as tc:
        with tc.tile_pool(name="const", bufs=1) as constp, \
             tc.tile_pool(name="dram", bufs=1, space="DRAM") as dram, \
             tc.tile_pool(name="resid", bufs=1) as resid:
            ones_f = constp.tile([P, P], F32)
            nc.any.memset(ones_f[:], 1.0)
            ones_b = constp.tile([P, P], BF16)
            nc.vector.tensor_copy(out=ones_b[:], in_=ones_f[:])
            eps_sb = constp.tile([P, 1], F32)
            nc.any.memset(eps_sb[:], EPS)
            cos_sb = constp.tile([P, tb], F32)
            sin_sb = constp.tile([P, tb], F32)

            # gathered normalized latents: slot s = quarter s of the batch
            q_all = dram.tile([4, P, QRC, tq], BF16)
            kv_all = dram.tile([4, P, KVRC, tq], BF16)
            if not sim_local:
                q_in = dram.tile([P, QRC, tq], BF16)
                kv_in = dram.tile([P, KVRC, tq], BF16)
            v_d = dram.tile([P, ntc, NH * DV], BF16)

            # SBUF-resident q/k intermediates (bf16): produced by B/C/k_rope,
            # consumed by attention.
            qnope_sb = resid.tile([P, NH, tb], BF16)
            qrope_sb = resid.tile([P, NH * DR // P, tb], BF16)
            knope_sb = resid.tile([P, NH, tb], BF16)
            krope_sb = resid.tile([P, NH * DR // P, tb], BF16)

            def rope_evict(lo_src, hi_src, tsl, pool, tag):
                """lo/hi chunk pair [P, TT] (4 heads x 32 rows) -> rotate."""
                t1 = pool.tile([P, TT], F32, tag=tag, name="rt1")
                t2 = pool.tile([P, TT], F32, tag=tag, name="rt2")
                nc.vector.tensor_mul(out=t1[:], in0=lo_src[:], in1=cos_sb[:, tsl])
                nc.vector.tensor_mul(out=t2[:], in0=hi_src[:], in1=sin_sb[:, tsl])
                lo_o = pool.tile([P, TT], BF16, tag=tag, name="rlo")
                nc.vector.tensor_sub(out=lo_o[:], in0=t1[:], in1=t2[:])
                t3 = pool.tile([P, TT], F32, tag=tag, name="rt3")
                t4 = pool.tile([P, TT], F32, tag=tag, name="rt4")
                nc.vector.tensor_mul(out=t3[:], in0=hi_src[:], in1=cos_sb[:, tsl])
                nc.vector.tensor_mul(out=t4[:], in0=lo_src[:], in1=sin_sb[:, tsl])
                hi_o = pool.tile([P, TT], BF16, tag=tag, name="rhi")
                nc.vector.tensor_add(out=hi_o[:], in0=t3[:], in1=t4[:])
                return lo_o, hi_o

            def rope_store(lo_o, hi_o, ci, dst_sb, tsl):
                for hh in range(4):
                    h = ci * 4 + hh
                    dc, dp = h // 2, 64 * (h % 2)
                    nc.sync.dma_start(
                        out=dst_sb[dp:dp + 32, dc, tsl],
                        in_=lo_o[32 * hh:32 * hh + 32, :])
                    nc.sync.dma_start(
                        out=dst_sb[dp + 32:dp + 64, dc, tsl],
                        in_=hi_o[32 * hh:32 * hh + 32, :])

            # ---------------- Phase A1: seq-parallel a-projections ----------
            def aproj(xsrc, q_dst, kv_dst):
                """a-projections for one token quarter; xsrc: [P, HIDC, tq]
                chunk AP; writes normalized latents to q_dst/kv_dst."""
                with tc.tile_pool(name="axq", bufs=HIDC + 1) as xqp, \
                     tc.tile_pool(name="apw", bufs=4) as wpool, \
                     tc.tile_pool(name="araw", bufs=QRC + KVRC + 1) as rawp, \
                     tc.tile_pool(name="aev", bufs=8) as evp, \
                     tc.tile_pool(name="aacc", bufs=7, space="PSUM") as accp, \
                     tc.tile_pool(name="astat", bufs=1, space="PSUM") as statp:
                    xq = []
                    for k in range(HIDC):
                        xt = xqp.tile([P, tq], BF16, tag="axq", name=f"axq{k}")
                        nc.sync.dma_start(out=xt[:], in_=xsrc[:, k, :])
                        xq.append(xt)

                    def proj_kind(kind, src, nrank, groups, dst):
                        nchunk = nrank // P
                        stat = statp.tile([P, tq], F32, tag="stat",
                                          name=f"stat_{kind}")
                        raws = []
                        for (m0, m1) in groups:
                            nm = m1 - m0
                            accs = [accp.tile([P, tq], F32, tag="acc",
                                              name=f"acc_{kind}{m}")
                                    for m in range(m0, m1)]
                            for k in range(HIDC):
                                wt = wpool.tile([P, nm * P], BF16, tag="apw",
                                                name=f"apw_{kind}{m0}_{k}")
                                nc.sync.dma_start(out=wt[:],
                                                  in_=src[:, k, m0 * P:m1 * P])
                                for mi in range(nm):
                                    nc.tensor.matmul(
                                        accs[mi][:], wt[:, mi * P:(mi + 1) * P],
                                        xq[k][:], start=(k == 0),
                                        stop=(k == HIDC - 1))
                            for mi, m in enumerate(range(m0, m1)):
                                raw = rawp.tile([P, tq], BF16, tag="araw",
                                                name=f"raw_{kind}{m}")
                                nc.vector.tensor_copy(out=raw[:],
                                                      in_=accs[mi][:])
                                raws.append(raw)
                                sq = evp.tile([P, tq], BF16, tag="aev")
                                nc.vector.tensor_mul(out=sq[:], in0=raw[:],
                                                     in1=raw[:])
                                nc.tensor.matmul(stat[:], ones_b[:], sq[:],
                                                 start=(m == 0),
                                                 stop=(m == nchunk - 1))
                        sdev = evp.tile([P, tq], F32, tag="aev")
                        nc.scalar.activation(sdev[:], stat[:], SQRT_FN,
                                             bias=eps_sb[:], scale=1.0 / nrank)
                        rstd = evp.tile([P, tq], F32, tag="aev",
                                        name=f"rstd_{kind}")
                        nc.vector.reciprocal(rstd[:], sdev[:])
                        for m in range(nchunk):
                            o = evp.tile([P, tq], BF16, tag="aev")
                            nc.vector.tensor_mul(out=o[:], in0=raws[m][:],
                                                 in1=rstd[:])
                            nc.sync.dma_start(out=dst[:, m, :], in_=o[:])

                    proj_kind("kv", kva_ap, KVR, [(0, 4)], kv_dst)
                    if not sim_local:
                        nc.gpsimd.collective_compute(
                            "AllGather", mybir.AluOpType.bypass,
                            replica_groups=RGROUPS,
                            ins=[kv_in.opt()], outs=[kv_all.opt()])
                    proj_kind("q", qa_ap, QR, [(0, 6), (6, 12)], q_dst)
                    if not sim_local:
                        nc.gpsimd.collective_compute(
                            "AllGather", mybir.AluOpType.bypass,
                            replica_groups=RGROUPS,
                            ins=[q_in.opt()], outs=[q_all.opt()])

            if sim_local:
                for s in range(4):
                    aproj(x_ap[:, :, s * tq:(s + 1) * tq],
                          q_all[s, :, :, :], kv_all[s, :, :, :])
            else:
                aproj(xq_ap[:, :, :], q_in[:, :, :], kv_in[:, :, :])

            # cos/sin needed from k_rope onwards; emitted after aproj so the
            # startup DMA burst feeds the a-projections first.
            nc.sync.dma_start(out=cos_sb[:], in_=cos_in[:, :])
            nc.sync.dma_start(out=sin_sb[:], in_=sin_in[:, :])

            # ---------------- Phase A2: k_rope over all tokens --------------
            with tc.tile_pool(name="krw", bufs=1) as krwp, \
                 tc.tile_pool(name="krx", bufs=4) as xpool, \
                 tc.tile_pool(name="krev", bufs=10) as evp, \
                 tc.tile_pool(name="kracc", bufs=4, space="PSUM") as accp:
                krw = krwp.tile([P, HIDC, NH * DR], BF16)
                nc.sync.dma_start(out=krw[:], in_=kr_ap[:, :, :])
                for t in range(ntt):
                    tsl = slice(t * TT, (t + 1) * TT)
                    accs = [accp.tile([P, TT], F32, tag="acc",
                                      name=f"kracc{t}_{m}")
                            for m in range(4)]
                    for k in range(HIDC):
                        xt = xpool.tile([P, TT], BF16, tag="krx",
                                        name=f"krx{t}_{k}")
                        nc.sync.dma_start(out=xt[:], in_=x_ap[:, k, tsl])
                        for mi in range(4):
                            nc.tensor.matmul(
                                accs[mi][:], krw[:, k, mi * P:(mi + 1) * P],
                                xt[:], start=(k == 0), stop=(k == HIDC - 1))
                    for ci in range(2):
                        lo_o, hi_o = rope_evict(accs[ci], accs[2 + ci], tsl,
                                                evp, "krev")
                        rope_store(lo_o, hi_o, ci, krope_sb, tsl)

            # ---------------- Phase C: kv_b (k_nope + v) --------------------
            # runs before q_b: it only needs the (earlier) kv AllGather.
            nspt = TT // tq if TT > tq else 1  # gathered slots per token tile
            with tc.tile_pool(name="kvw", bufs=1) as kvwp, \
                 tc.tile_pool(name="kvn", bufs=2) as kvnp, \
                 tc.tile_pool(name="kvev", bufs=6) as evp, \
                 tc.tile_pool(name="kvps", bufs=4, space="PSUM") as kvps:
                kbw = kvwp.tile([P, KVRC, NH * DN], BF16)
                vbw = kvwp.tile([P, KVRC, NH * DV], BF16)
                nc.sync.dma_start(out=kbw[:], in_=kvbk_ap[:, :, :])
                nc.sync.dma_start(out=vbw[:], in_=kvbv_ap[:, :, :])
                for t in range(ntt):
                    tsl = slice(t * TT, (t + 1) * TT)
                    kvn = kvnp.tile([P, KVRC, TT], BF16, tag="kvn",
                                    name=f"kvn{t}")
                    for si in range(nspt):
                        s = (t * TT) // tq + si
                        nc.sync.dma_start(
                            out=kvn[:, :, si * tq:(si + 1) * tq]
                            if nspt > 1 else kvn[:, :, :],
                            in_=kv_all[s, :, :, :])
                    for m in range(NH):
                        ps = kvps.tile([P, TT], F32, tag="kps",
                                       name=f"kb{t}_{m}")
                        for k in range(KVRC):
                            nc.tensor.matmul(ps[:], kbw[:, k, m * P:(m + 1) * P],
                                             kvn[:, k, :], start=(k == 0),
                                             stop=(k == KVRC - 1))
                        nc.vector.tensor_copy(out=knope_sb[:, m, tsl],
                                              in_=ps[:])
                    for tc8 in range(TT // P):
                        for vc in range(NH * DV // TT):
                            ps = kvps.tile([P, TT], F32, tag="vps",
                                           name=f"v{t}_{tc8}_{vc}")
                            for k in range(KVRC):
                                nc.tensor.matmul(
                                    ps[:],
                                    kvn[:, k, tc8 * P:(tc8 + 1) * P],
                                    vbw[:, k, vc * TT:(vc + 1) * TT],
                                    start=(k == 0), stop=(k == KVRC - 1))
                            o = evp.tile([P, TT], BF16, tag="kvev")
                            nc.vector.tensor_copy(out=o[:], in_=ps[:])
                            nc.sync.dma_start(
                                out=v_d[:, t * (TT // P) + tc8,
                                        vc * TT:(vc + 1) * TT],
                                in_=o[:])

            # ---------------- Phase B: q_b + q rope -------------------------
            with tc.tile_pool(name="qbw", bufs=1) as qbwp, \
                 tc.tile_pool(name="qbn", bufs=2) as qnp, \
                 tc.tile_pool(name="qbev", bufs=10) as evp, \
                 tc.tile_pool(name="qbps", bufs=4, space="PSUM") as qbps:
                qbw = qbwp.tile([P, QRC, NH * (DN + DR)], BF16)
                nc.sync.dma_start(out=qbw[:], in_=qb_ap[:, :, :])
                for t in range(ntt):
                    tsl = slice(t * TT, (t + 1) * TT)
                    qn = qnp.tile([P, QRC, TT], BF16, tag="qn", name=f"qn{t}")
                    for si in range(nspt):
                        s = (t * TT) // tq + si
                        nc.sync.dma_start(
                            out=qn[:, :, si * tq:(si + 1) * tq]
                            if nspt > 1 else qn[:, :, :],
                            in_=q_all[s, :, :, :])
                    rope_ps = {}
                    for mg in range(3):
                        pss = []
                        for m in range(mg * 4, mg * 4 + 4):
                            ps = qbps.tile([P, TT], F32, tag="qbps",
                                           name=f"qb{t}_{m}")
                            for k in range(QRC):
                                nc.tensor.matmul(
                                    ps[:], qbw[:, k, m * P:(m + 1) * P],
                                    qn[:, k, :], start=(k == 0),
                                    stop=(k == QRC - 1))
                            pss.append(ps)
                        if mg < 2:
                            for mi, ps in enumerate(pss):
                                nc.vector.tensor_copy(
                                    out=qnope_sb[:, mg * 4 + mi, tsl],
                                    in_=ps[:])
                        else:
                            for mi, ps in enumerate(pss):
                                rope_ps[mi] = ps
                    for ci in range(2):
                        lo_o, hi_o = rope_evict(rope_ps[ci], rope_ps[2 + ci],
                                                tsl, evp, "qbev")
                        rope_store(lo_o, hi_o, ci, qrope_sb, tsl)

            # ---------------- Phase D: attention ----------------------------
            # q/k inputs read in place from the resident tiles; V streamed.
            nkt = tb // P  # key tiles
            with tc.tile_pool(name="aosb", bufs=1) as aop:
                attn_sb = aop.tile([P, NH, tb], BF16)
                with tc.tile_pool(name="ath", bufs=2) as hp, \
                     tc.tile_pool(name="atex", bufs=4) as exp_p, \
                     tc.tile_pool(name="atev", bufs=4) as evp, \
                     tc.tile_pool(name="atsc", bufs=3, space="PSUM") as scp, \
                     tc.tile_pool(name="atpv", bufs=2, space="PSUM") as pvp, \
                     tc.tile_pool(name="atden", bufs=2, space="PSUM") as denp:
                    for h in range(NH):
                        rp, rc = 64 * (h % 2), h // 2
                        v_h = hp.tile([P, nkt, DV], BF16, tag="vh",
                                      name=f"vh{h}")
                        nc.sync.dma_start(out=v_h[:],
                                          in_=v_d[:, :, h * DV:(h + 1) * DV])
                        for qt in range(ntt):
                            qsl = slice(qt * TT, (qt + 1) * TT)
                            pv_ps = pvp.tile([P, TT], F32, tag="pv",
                                             name=f"pv{h}_{qt}")
                            den_ps = denp.tile([P, TT], F32, tag="den",
                                               name=f"den{h}_{qt}")
                            for kt in range(nkt):
                                ksl = slice(kt * P, (kt + 1) * P)
                                sc_ps = scp.tile([P, TT], F32, tag="sc",
                                                 name=f"sc{h}_{qt}_{kt}")
                                nc.tensor.matmul(sc_ps[:],
                                                 knope_sb[:, h, ksl],
                                                 qnope_sb[:, h, qsl],
                                                 start=True, stop=False)
                                nc.tensor.matmul(
                                    sc_ps[:],
                                    krope_sb[rp:rp + 64, rc, ksl],
                                    qrope_sb[rp:rp + 64, rc, qsl],
                                    start=False, stop=True)
                                ex = exp_p.tile([P, TT], BF16, tag="ex",
                                                name=f"ex{h}_{qt}_{kt}")
                                nc.scalar.activation(ex[:], sc_ps[:], EXP_FN,
                                                     scale=SCALE)
                                nc.tensor.matmul(pv_ps[:], v_h[:, kt, :], ex[:],
                                                 start=(kt == 0),
                                                 stop=(kt == nkt - 1))
                                nc.tensor.matmul(den_ps[:], ones_b[:], ex[:],
                                                 start=(kt == 0),
                                                 stop=(kt == nkt - 1))
                            recip = evp.tile([P, TT], F32, tag="atev",
                                             name="recip")
                            nc.vector.reciprocal(recip[:], den_ps[:])
                            nc.vector.tensor_mul(out=attn_sb[:, h, qsl],
                                                 in0=pv_ps[:], in1=recip[:])

                # ------------ Phase E: o_proj (partial), n-outer ------------
                with tc.tile_pool(name="oww", bufs=3) as owp, \
                     tc.tile_pool(name="oev", bufs=4) as evp2, \
                     tc.tile_pool(name="ops", bufs=4, space="PSUM") as ops:
                    for n in range(HID // TT):
                        oww = owp.tile([P, NH * DV // P, TT], BF16, tag="oww",
                                       name=f"oww{n}")
                        nc.sync.dma_start(
                            out=oww[:], in_=ow_ap[:, :, n * TT:(n + 1) * TT])
                        for t8 in range(ntc):
                            ps = ops.tile([P, TT], F32, tag="ops",
                                          name=f"o{n}_{t8}")
                            for k in range(NH * DV // P):
                                nc.tensor.matmul(
                                    ps[:],
                                    attn_sb[:, k, t8 * P:(t8 + 1) * P],
                                    oww[:, k, :],
                                    start=(k == 0),
                                    stop=(k == NH * DV // P - 1))
                            o = evp2.tile([P, TT], F32, tag="oev")
                            nc.vector.tensor_copy(out=o[:], in_=ps[:])
                            nc.sync.dma_start(
                                out=out_part[t8 * P:(t8 + 1) * P,
                                             n * TT:(n + 1) * TT],
                                in_=o[:])

    nc.compile()
    return nc


# ---------------------------------------------------------------------------
# host-side packing
# ---------------------------------------------------------------------------

def _bf16(x):
    import ml_dtypes

    return np.asarray(x, dtype=ml_dtypes.bfloat16)


def _rope_tables():
    inv_freq = 1.0 / (10000.0 ** (np.arange(0, DR, 2, dtype=np.float32) / DR))
    t = np.arange(MAXP, dtype=np.float32)
    freqs = np.outer(t, inv_freq)
    emb = np.concatenate([freqs, freqs], axis=-1)
    return np.cos(emb).astype(np.float32), np.sin(emb).astype(np.float32)


def core_weights(g, q_a_w, q_a_ln_w, q_b_w, kv_a_w, kv_a_ln_w, kv_b_w,
                 k_rope_w, o_w):
    """Pack the weight set for head-group g (heads g*8 .. g*8+8)."""
    heads = range(g * NH, (g + 1) * NH)
    qb_eff = (q_b_w * q_a_ln_w[None, :]).astype(np.float32)
    kvb_eff = (kv_b_w * kv_a_ln_w[None, :]).astype(np.float32)

    nope_rows = np.concatenate(
        [np.arange(h * (DN + DR), h * (DN + DR) + DN) for h in heads])
    lo_rows = np.concatenate(
        [np.arange(h * (DN + DR) + DN, h * (DN + DR) + DN + 32) for h in heads])
    hi_rows = np.concatenate(
        [np.arange(h * (DN + DR) + DN + 32, h * (DN + DR) + DN + 64)
         for h in heads])
    qb_rows = np.concatenate([nope_rows, lo_rows, hi_rows])

    k_rows = np.concatenate(
        [np.arange(h * (DN + DV), h * (DN + DV) + DN) for h in heads])
    v_rows = np.concatenate(
        [np.arange(h * (DN + DV) + DN, (h + 1) * (DN + DV)) for h in heads])

    kr_lo = np.concatenate([np.arange(h * DR, h * DR + 32) for h in heads])
    kr_hi = np.concatenate([np.arange(h * DR + 32, (h + 1) * DR) for h in heads])
    kr_rows = np.concatenate([kr_lo, kr_hi])

    o_cols = np.concatenate([np.arange(h * DV, (h + 1) * DV) for h in heads])

    c = np.ascontiguousarray
    return {
        "qa_wT": _bf16(c(q_a_w.T)),
        "kva_wT": _bf16(c(kv_a_w.T)),
        "kr_wT": _bf16(c(k_rope_w[kr_rows].T)),
        "qb_wT": _bf16(c(qb_eff[qb_rows].T)),
        "kvbk_wT": _bf16(c(kvb_eff[k_rows].T)),
        "kvbv_wT": _bf16(c(kvb_eff[v_rows].T)),
        "o_wT": _bf16(c(o_w[:, o_cols].T)),
    }


def core_inputs(c, hidden_states, position_ids, weight_sets, tb=S,
                sim_local=False):
    b, g = c // 4, c % 4
    tq = tb // 4
    cos_tab, sin_tab = _rope_tables()
    pos = np.asarray(position_ids[b][:tb])
    cos_b = cos_tab[pos]  # [tb, DR]
    sin_b = sin_tab[pos]
    cos_rep = np.ascontiguousarray(np.tile(cos_b[:, :32].T, (4, 1)))
    sin_rep = np.ascontiguousarray(np.tile(sin_b[:, :32].T, (4, 1)))
    x = np.asarray(hidden_states[b][:tb], dtype=np.float32)
    xT = _bf16(np.ascontiguousarray(x.T))
    im = {"xT": xT, "cos_rep": cos_rep, "sin_rep": sin_rep}
    if not sim_local:
        im["xqT"] = np.ascontiguousarray(xT[:, g * tq:(g + 1) * tq])
    im.update(weight_sets[g])
    return im


_CACHE = {}


def _get_nc(tb=S):
    if tb not in _CACHE:
        _CACHE[tb] = build_nc(tb)
    return _CACHE[tb]


def kernel(hidden_states, position_ids, q_a_w, q_a_ln_w, q_b_w,
           kv_a_w, kv_a_ln_w, kv_b_w, k_rope_w, o_w):
    hidden_states = np.asarray(hidden_states, dtype=np.float32)
    weight_sets = [
        core_weights(g, np.asarray(q_a_w, np.float32),
                     np.asarray(q_a_ln_w, np.float32),
                     np.asarray(q_b_w, np.float32),
                     np.asarray(kv_a_w, np.float32),
                     np.asarray(kv_a_ln_w, np.float32),
                     np.asarray(kv_b_w, np.float32),
                     np.asarray(k_rope_w, np.float32),
                     np.asarray(o_w, np.float32))
        for g in range(4)
    ]
    in_maps = [core_inputs(c, hidden_states, position_ids, weight_sets)
               for c in range(NCORES)]
    nc = _get_nc()
    res = run_bass_kernel_spmd(nc, in_maps, core_ids=list(range(NCORES)))
    out = np.zeros((B, S, HID), dtype=np.float64)
    for c in range(NCORES):
        out[c // 4] += res.results[c]["out_part"]
    return out.astype(np.float32)
